# revision 14
# baseline (speedup 1.0000x reference)
"""Trainium2 Bass kernel for single-head causal attention (B=4, T=4096, C=2048, HS=128).

Sharding: 2 cores per batch element (8 cores, B=4), interleaved 512-row
q-chunks: role A (even cores) owns global chunks [0,2,4,6], role B (odd)
[1,3,5,7]. Each core projects Q^T/K^T/V^T for its own 2048 rows, AllGathers
K^T/V^T within its pair via DRAM staging, and runs causal attention over
its 4 q-slots with static per-slot extents of 2(s+1) 512-chunks.

Attention is split own-chunks-first / partner-chunks-second so the
AllGather overlaps the Q projections + the own half of attention. Scores
for 3 k-tiles at a time land in a 3-bank PSUM tile and are exponentiated
by one batched ACT instruction; AV partials and column-packed
(tile_position) denominator partials accumulate per group in a 2-bank
PSUM tile and are folded into per-slot SBUF accumulators by the vector
engine, with the AV matmuls lagging the score matmuls by two groups so
the exp stream and the PE stream overlap. Softmax normalization uses a
broadcast matmul (ones/32 stationary) + reciprocal_approx_fast. The
output is y^T per slot, transposed on the host. Partner addressing is
role-independent: both parities of the gathered buffer are blended with
per-core 0/1 selectors, so all 8 cores run one SPMD graph.
"""

import math
from collections import deque

import numpy as np
import ml_dtypes

import concourse.bacc as bacc
import concourse.tile as tile
from concourse import mybir
from concourse.bass_utils import run_bass_kernel_spmd

B, T, C, HS = 4, 4096, 2048, 128
NCORES = 8
TOWN = 2048              # sequence rows owned per core
NCT = C // 128           # 16 contraction tiles
QTILES_A = [0, 2, 4, 6]
QTILES_B = [1, 3, 5, 7]

BF16 = ml_dtypes.bfloat16


def build_graph(with_collective=True, sbuf_transpose=True):
    nc = bacc.Bacc(
        "TRN2", target_bir_lowering=False, debug=False, num_devices=NCORES
    )
    bf = mybir.dt.bfloat16
    f32 = mybir.dt.float32
    EXP = mybir.ActivationFunctionType.Exp

    xt_d = nc.dram_tensor("xt", [128, NCT, TOWN], bf, kind="ExternalInput")
    w3_d = nc.dram_tensor("w3", [128, 3, NCT, HS], bf, kind="ExternalInput")
    mo_d = nc.dram_tensor("mo", [128, 2048], bf, kind="ExternalInput")
    ps0_d = nc.dram_tensor("ps0", [128, 1], f32, kind="ExternalInput")
    ps1_d = nc.dram_tensor("ps1", [128, 1], f32, kind="ExternalInput")
    psz_d = nc.dram_tensor("psz", [128, 1], f32, kind="ExternalInput")
    # y^T per slot, normalized; host transposes to [512, HS]
    out_d = nc.dram_tensor("out", [4, 128, 512], f32, kind="ExternalOutput")

    with tile.TileContext(nc) as tc:
        with (
            tc.tile_pool(name="big", bufs=1) as big,
            tc.tile_pool(name="dram", bufs=1, space="DRAM") as dram,
        ):
            # ---- loads: x c-tiles round-robin in consumption order;
            # w3 pieces and small tensors interleaved to balance queues ----
            w3 = big.tile([128, 3, NCT, HS], bf, tag="w3")
            xt = big.tile([128, NCT, TOWN], bf, tag="xt")

            nc.scalar.dma_start(w3[:, 1:3, 0:4, :], w3_d[:, 1:3, 0:4, :])
            qs = [nc.sync, nc.scalar, nc.gpsimd]
            for c in range(NCT):
                qs[c % 3].dma_start(xt[:, c : c + 1, :], xt_d[:, c : c + 1, :])
                if c == 4:
                    nc.scalar.dma_start(
                        w3[:, 1:3, 4:10, :], w3_d[:, 1:3, 4:10, :]
                    )
                elif c == 10:
                    nc.scalar.dma_start(
                        w3[:, 1:3, 10:16, :], w3_d[:, 1:3, 10:16, :]
                    )
            ps0 = big.tile([128, 1], f32, tag="ps0")
            nc.gpsimd.dma_start(ps0[:], ps0_d[:])
            ps1 = big.tile([128, 1], f32, tag="ps1")
            nc.gpsimd.dma_start(ps1[:], ps1_d[:])
            psz = big.tile([128, 1], f32, tag="psz")
            nc.gpsimd.dma_start(psz[:], psz_d[:])
            nc.gpsimd.dma_start(w3[:, 0:1, :, :], w3_d[:, 0:1, :, :])
            mo = big.tile([128, 2048], bf, tag="mo")
            nc.gpsimd.dma_start(mo[:], mo_d[:])

            # ---- constants ----
            wut = big.tile([128, 512], bf, tag="wut")
            nc.vector.memset(wut[:], 0.0)
            on32 = big.tile([128, 32], bf, tag="on32")
            nc.vector.memset(on32[:], 1.0)
            scb = big.tile([128, 128], f32, tag="scb")
            nc.vector.memset(scb[:], 1.0 / 32.0)

            # ---- persistent SBUF tensors ----
            ktq = big.tile([128, TOWN], bf, tag="ktq")  # own K^T (slot order)
            vtq = big.tile([128, TOWN], bf, tag="vtq")  # own V^T
            v3o = big.tile([128, 16, HS], bf, tag="v3o")  # own V (k-major)
            qts = [
                big.tile([128, 512], bf, tag=f"qt{s}", name=f"qt{s}")
                for s in range(4)
            ]
            ktp = [
                big.tile([128, TOWN], bf, tag=f"ktp{r}", name=f"ktp{r}")
                for r in range(2)
            ]
            v3p = [
                big.tile([128, 16, HS], bf, tag=f"v3p{r}", name=f"v3p{r}")
                for r in range(2)
            ]
            ktpar = big.tile([128, TOWN], bf, tag="ktpar")
            v3par = big.tile([128, 16, HS], bf, tag="v3par")
            accs = [
                big.tile([128, 1024], f32, tag=f"acc{s}", name=f"acc{s}")
                for s in range(4)
            ]

            kvb = dram.tile([256, TOWN], bf, tag="kvb")
            kvg = dram.tile([512, TOWN], bf, tag="kvg")
            vst = dram.tile([128, TOWN], bf, tag="vst")

            # ---- projections: K,V then Q for own rows ----
            with tc.tile_pool(name="pjps", bufs=8, space="PSUM") as pjps:
                # PE warm-up while the first input DMAs land
                wup = pjps.tile([128, 512], f32, tag="pj", name="wup")
                for _ in range(28):
                    nc.tensor.matmul(
                        wup[:], wut[:, 0:128], wut[:], start=True, stop=True
                    )
                ps8 = [
                    pjps.tile([128, 512], f32, tag="pj", name=f"pa{i}")
                    for i in range(8)
                ]
                for c in range(NCT):
                    for wi in range(2):  # 0 = K, 1 = V
                        for t4 in range(4):
                            nc.tensor.matmul(
                                ps8[wi * 4 + t4][:],
                                w3[:, wi + 1, c, :],
                                xt[:, c, t4 * 512 : (t4 + 1) * 512],
                                start=(c == 0),
                                stop=(c == NCT - 1),
                            )
                for wi, dest in [(0, ktq), (1, vtq)]:
                    for t4 in range(4):
                        nc.vector.tensor_copy(
                            dest[:, t4 * 512 : (t4 + 1) * 512],
                            ps8[wi * 4 + t4][:],
                        )
                # staging writes on two queues in parallel
                nc.sync.dma_start(kvb[0:128, :], ktq[:])
                nc.scalar.dma_start(kvb[128:256, :], vtq[:])
                if sbuf_transpose:
                    nc.sync.dma_start_transpose(v3o[:], vtq[:])
                else:
                    nc.scalar.dma_start(vst[:], vtq[:])
                    nc.scalar.dma_start_transpose(v3o[:], vst[:])
                if with_collective:
                    nc.gpsimd.collective_compute(
                        "AllGather",
                        mybir.AluOpType.bypass,
                        replica_groups=[[0, 1], [2, 3], [4, 5], [6, 7]],
                        ins=[kvb.opt()],
                        outs=[kvg.opt()],
                    )
                else:  # timeline-model stub: same data volume, no comms
                    nc.scalar.dma_start(kvg[0:256, :], kvb[:])
                    nc.scalar.dma_start(kvg[256:512, :], kvb[:])
                # partner halves (block on the collective; tail of queues)
                nc.gpsimd.dma_start(ktp[0][:], kvg[0:128, :])
                nc.gpsimd.dma_start(ktp[1][:], kvg[256:384, :])
                nc.sync.dma_start_transpose(v3p[0][:], kvg[128:256, :])
                nc.sync.dma_start_transpose(v3p[1][:], kvg[384:512, :])

                # Q projections for all 4 slots (reuses the pj PSUM ring)
                pq = [
                    pjps.tile([128, 512], f32, tag="pj", name=f"pq{s}")
                    for s in range(4)
                ]
                for c in range(NCT):
                    for s in range(4):
                        nc.tensor.matmul(
                            pq[s][:],
                            w3[:, 0, c, :],
                            xt[:, c, s * 512 : (s + 1) * 512],
                            start=(c == 0),
                            stop=(c == NCT - 1),
                        )
                for s in range(4):
                    nc.vector.tensor_copy(qts[s][:], pq[s][:])

            # role-independent partner buffers: blend the two parities with
            # per-core 0/1 selectors (ps0 = partner-is-parity-0). Emitted
            # between the own and partner phases so the vector queue is not
            # head-of-line blocked on the collective during the own phase.
            def emit_blends():
                nc.vector.tensor_scalar_mul(ktp[0][:], ktp[0][:], ps0[:])
                nc.vector.tensor_scalar_mul(ktp[1][:], ktp[1][:], ps1[:])
                nc.vector.tensor_add(ktpar[:], ktp[0][:], ktp[1][:])
                nc.vector.tensor_scalar_mul(v3p[0][:], v3p[0][:], ps0[:])
                nc.vector.tensor_scalar_mul(v3p[1][:], v3p[1][:], ps1[:])
                nc.vector.tensor_add(v3par[:], v3p[0][:], v3p[1][:])

            # ---- attention ----
            with (
                tc.tile_pool(name="srng", bufs=2, space="PSUM") as srng,
                tc.tile_pool(name="pps", bufs=1, space="PSUM") as pps,
                tc.tile_pool(name="pp", bufs=4) as pp,
                tc.tile_pool(name="ep", bufs=2) as ep,
            ):
                def groups_of(ntiles):
                    out, i = [], 0
                    while i < ntiles:
                        n = min(3, ntiles - i)
                        out.append((i, n))
                        i += n
                    return out

                # scores + batched exp (+ causal mask) for one group
                def emit_scores(s, g0, n, own, mask_lo):
                    kt = ktq if own else ktpar
                    S = srng.tile([128, 1536], f32, tag="r", name="sg")
                    for i in range(n):
                        t = g0 + i
                        nc.tensor.matmul(
                            S[:, i * 512 : (i + 1) * 512],
                            kt[:, t * 128 : (t + 1) * 128],
                            qts[s][:],
                            start=True,
                            stop=True,
                        )
                    p = pp.tile([128, 1536], bf, tag="p", name="pg")
                    nc.scalar.activation(p[:, 0 : n * 512], S[:, 0 : n * 512], EXP)
                    # own diag chunk: structural mask; partner last chunk:
                    # per-core scalar (zero for role A, one for role B)
                    lo = max(g0, mask_lo) - g0
                    if g0 + n > mask_lo:
                        sl = p[:, lo * 512 : n * 512]
                        if own:
                            j = g0 + lo - mask_lo
                            nc.vector.tensor_mul(
                                sl, sl, mo[:, j * 512 : (j + n - lo) * 512]
                            )
                        else:
                            nc.vector.tensor_scalar_mul(sl, sl, psz[:])
                    return p

                # AV + column-packed denominator partials for one group
                def emit_av(s, g0, n, own, p, first):
                    v3 = v3o if own else v3par
                    P = pps.tile([128, 1024], f32, tag="pv", name="pv")
                    for i in range(n):
                        t = g0 + i
                        nc.tensor.matmul(
                            P[:, 0:512],
                            v3[:, t, :],
                            p[:, i * 512 : (i + 1) * 512],
                            start=(i == 0),
                            stop=(i == n - 1),
                            skip_group_check=True,
                        )
                    for i in range(4):
                        mv = (
                            p[:, i * 512 : (i + 1) * 512]
                            if i < n
                            else wut[:, 0:512]
                        )
                        nc.tensor.matmul(
                            P[32 * i : 32 * i + 32, 512:1024],
                            on32[:],
                            mv,
                            start=True,
                            stop=True,
                            skip_group_check=True,
                            tile_position=(0, 32 * i),
                        )
                    if first:
                        nc.vector.tensor_copy(accs[s][:], P[:, 0:1024])
                    else:
                        nc.vector.tensor_add(accs[s][:], accs[s][:], P[:, 0:1024])

                # software pipeline, AV lagging scores by two groups
                pending = deque()

                def emit_phase(s, own):
                    ntiles = 4 * (s + 1)
                    mask_lo = ntiles - 4
                    first = own
                    for g0, n in groups_of(ntiles):
                        p = emit_scores(s, g0, n, own, mask_lo)
                        if len(pending) >= 2:
                            emit_av(*pending.popleft())
                        pending.append((s, g0, n, own, p, first and g0 == 0))

                def drain():
                    while pending:
                        emit_av(*pending.popleft())

                for s in range(4):
                    emit_phase(s, True)
                emit_blends()
                for s in (3, 2, 1, 0):
                    emit_phase(s, False)
                    drain()  # acc[s] complete before its epilogue
                    FD = srng.tile([128, 1536], f32, tag="r", name=f"fd{s}")
                    nc.tensor.matmul(
                        FD[:, 0:512],
                        scb[:],
                        accs[s][:, 512:1024],
                        start=True,
                        stop=True,
                    )
                    fdc = ep.tile([128, 512], f32, tag="fdc", name=f"fdc{s}")
                    nc.vector.tensor_copy(fdc[:], FD[:, 0:512])
                    rb = ep.tile([128, 512], f32, tag="rb", name=f"rb{s}")
                    nc.vector.reciprocal_approx_fast(rb[:], fdc[:])
                    ot = ep.tile([128, 512], f32, tag="ot", name=f"ot{s}")
                    nc.vector.tensor_mul(ot[:], accs[s][:, 0:512], rb[:])
                    nc.gpsimd.dma_start(out_d[s], ot[:])

    nc.compile()
    return nc


def _role_qtiles(h):
    return QTILES_A if h == 0 else QTILES_B


def _diag_mask():
    """[128, 4*512] bf16: tile j of the diagonal 512-chunk, k<=q."""
    m = np.zeros((128, 4, 512), np.float32)
    k = np.arange(128)[:, None]
    q = np.arange(512)[None, :]
    for j in range(4):
        m[:, j, :] = (128 * j + k <= q).astype(np.float32)
    return np.ascontiguousarray(m.reshape(128, 2048)).astype(BF16)


def make_in_maps(x, Wq, Wk, Wv):
    """Host-side sharding + layout prep. x [B,T,C] f32, W* [C,HS] f32."""
    wq_s = np.asarray(Wq, np.float32) / math.sqrt(HS)
    w3 = np.stack(
        [wq_s, np.asarray(Wk, np.float32), np.asarray(Wv, np.float32)]
    )
    w3_arr = np.ascontiguousarray(
        w3.reshape(3, NCT, 128, HS).transpose(2, 0, 1, 3)
    ).astype(BF16)
    mo = _diag_mask()

    in_maps = []
    for core in range(NCORES):
        b, h = core // 2, core % 2
        qtiles = _role_qtiles(h)
        rows = np.concatenate(
            [np.arange(g * 512, (g + 1) * 512) for g in qtiles]
        )
        xr = np.asarray(x[b])[rows]  # [2048 rows, C] f32
        xT = np.ascontiguousarray(xr.T).astype(BF16)  # [C, 2048]
        xt_arr = np.ascontiguousarray(
            xT.reshape(NCT, 128, TOWN).transpose(1, 0, 2)
        )  # [128, NCT, 2048]
        in_maps.append(
            {
                "xt": xt_arr,
                "w3": w3_arr,
                "mo": mo,
                # partner parity selectors: partner parity = 1-h
                "ps0": np.full((128, 1), float(h), np.float32),
                "ps1": np.full((128, 1), float(1 - h), np.float32),
                # partner last chunk: fully masked for role A, visible for B
                "psz": np.full((128, 1), float(h), np.float32),
            }
        )
    return in_maps


def assemble_out(results):
    """results: list of 8 dicts with 'out' [4,128,512] -> y [B,T,HS] f32."""
    y = np.zeros((B, T, HS), np.float32)
    for core in range(NCORES):
        b, h = core // 2, core % 2
        qtiles = _role_qtiles(h)
        o = np.asarray(results[core]["out"])  # [4, 128, 512] = y^T per slot
        for s in range(4):
            g = qtiles[s]
            y[b, g * 512 : (g + 1) * 512] = o[s].T
    return y


_NC_CACHE = None


def _get_graph():
    global _NC_CACHE
    if _NC_CACHE is None:
        _NC_CACHE = build_graph()
    return _NC_CACHE


def kernel(x, Wq, Wk, Wv):
    import time

    nc = _get_graph()
    in_maps = make_in_maps(x, Wq, Wk, Wv)
    try:
        res = run_bass_kernel_spmd(nc, in_maps, list(range(NCORES)))
    except Exception:
        time.sleep(15)  # transient device/mesh hiccup: one retry
        res = run_bass_kernel_spmd(nc, in_maps, list(range(NCORES)))
    return assemble_out(res.results)
